# revision 1
# baseline (speedup 1.0000x reference)
"""CrossAttention Trainium2 kernel.

Full inputs in, full outputs out. Sharding: data-parallel over batch
(B=8 -> 1 batch element per NeuronCore, 8 cores, no collectives).

Per-core math (N=1024, DIM=768, H=12, HD=64):
    kv  = en @ Wkv + bkv                    (k = kv[:, :768], v = kv[:, 768:])
    s_h = (q_h @ k_h^T) * HD^-0.5           per head h
    a_h = softmax(s_h, axis=k)              (max-subtraction skipped; |s|<5)
    wa  = concat_h(a_h @ v_h)
    out = wa @ Wproj + bproj

Layout strategy: everything that enters a matmul contraction is kept
contraction-on-partitions.  en/dec are transposed once on the PE; k is
produced directly transposed (kT[d, n]); the attention matrix is produced
transposed (scoresT[k, q]) so that exp(scoresT) can feed the wa matmul with
no further transposes.  A ones-column appended to v gives the softmax row
sums as a 65th output row of the waT matmul.  Normalization happens on the
waT tiles (64 x q) via a one-hot broadcast matmul + DVE multiply.
"""

import sys

import numpy as np

if "/opt/trn_rl_repo" not in sys.path:
    sys.path.insert(0, "/opt/trn_rl_repo")

import concourse.bass as bass
import concourse.mybir as mybir
import concourse.tile as tile
from concourse.bass_utils import run_bass_kernel_spmd

B, N, DIM, H, HD = 8, 1024, 768, 12, 64
NCORES = 8
SCALE = float(HD) ** -0.5
F32 = mybir.dt.float32

# Matmul streaming dtype: float32r streams 4-byte floats at 1 cycle/row for
# free-dim >= 256 (plain float32 is 4 cycles/row).
MM_DT = mybir.dt.float32r

NC6 = DIM // 128  # 6 contraction chunks of 128
NP = H // 2       # 6 head pairs (head pair p = heads 2p, 2p+1)
NN8 = N // 128    # 8 row chunks of 128
NQ2 = N // 512    # 2 q tiles of 512


def _r(ap):
    return ap.bitcast(MM_DT)


def build_bass():
    nc = bass.Bass()
    en_d = nc.declare_dram_parameter("en", [N, DIM], F32, isOutput=False)
    dec_d = nc.declare_dram_parameter("dec", [N, DIM], F32, isOutput=False)
    wkv_d = nc.declare_dram_parameter("Wkv", [DIM, 2 * DIM], F32, isOutput=False)
    wproj_d = nc.declare_dram_parameter("Wproj", [DIM, DIM], F32, isOutput=False)
    ident_d = nc.declare_dram_parameter("ident", [128, 128], F32, isOutput=False)
    bkvk_d = nc.declare_dram_parameter("bkv_k", [128, NC6], F32, isOutput=False)
    bkvv_d = nc.declare_dram_parameter("bkv_v", [128, DIM], F32, isOutput=False)
    bproj_d = nc.declare_dram_parameter("bproj_b", [128, DIM], F32, isOutput=False)
    out_d = nc.declare_dram_parameter("out", [N, DIM], F32, isOutput=True)

    with tile.TileContext(nc) as tc:
        with (
            tc.tile_pool(name="const", bufs=1) as cp,
            tc.tile_pool(name="big", bufs=1) as bp,
            tc.tile_pool(name="io", bufs=3) as io,
            tc.tile_pool(name="absp", bufs=1, space="PSUM") as absp,
            nc.allow_low_precision(reason="fp32r matmul streaming (tf32-like)"),
        ):
            abs_t = absp.tile([1, 8], F32, tag="abs")
            abs_state = {"first": True}

            def absorb(src_ap):
                nc.tensor.matmul(
                    abs_t[:], src_ap[0:1, 0:1], src_ap[0:1, 0:8],
                    start=abs_state["first"], stop=False,
                    skip_group_check=True,
                )
                abs_state["first"] = False
            # ---- constants -------------------------------------------------
            ident = cp.tile([128, 128], F32, tag="ident")
            nc.sync.dma_start(ident[:], ident_d[:])
            bkvk = cp.tile([128, NC6], F32, tag="bkvk")
            nc.sync.dma_start(bkvk[:], bkvk_d[:])
            bkvv = cp.tile([128, DIM], F32, tag="bkvv")
            nc.sync.dma_start(bkvv[:], bkvv_d[:])
            bprojb = cp.tile([128, DIM], F32, tag="bprojb")
            nc.sync.dma_start(bprojb[:], bproj_d[:])

            # ---- persistent big tensors -----------------------------------
            # xT layout: [128, 6*1024]; col block cj*1024+n holds x[n, cj*128+p].
            decT = bp.tile([128, NC6 * N], F32, tag="decT")
            kT = bp.tile([128, NC6 * N], F32, tag="kT")
            waT = bp.tile([128, NC6 * N], F32, tag="waT")
            # v_ext[kc]: [128, 780]; head h at cols 65h..65h+63, ones at 65h+64
            vext = [
                bp.tile([128, H * (HD + 1)], F32, tag=f"vext{i}", name=f"vext{i}")
                for i in range(NN8)
            ]
            ones64 = bp.tile([1, HD], F32, tag="ones64")
            o64f = bp.tile([1, HD], F32, tag="o64f")
            nc.vector.memset(o64f[:], 1.0)
            nc.vector.tensor_copy(_r(ones64[:]), o64f[:])
            onesrc = bp.tile([128, H], F32, tag="onesrc")
            nc.vector.memset(onesrc[:], 1.0)

            # =================================================================
            # Phase A: transpose en, dec  ([n, c] -> [c, n])
            # =================================================================
            with tc.tile_pool(name="abE", bufs=1) as abE:
                enT = abE.tile([128, NC6 * N], F32, tag="enT")
                with (
                    tc.tile_pool(name="stg", bufs=1) as sp,
                    tc.tile_pool(name="tp", bufs=2, space="PSUM") as tpp,
                ):
                    absorb(ident)
                    stages = []
                    for tag, src_d in (("enstg", en_d), ("decstg", dec_d)):
                        stg = sp.tile([128, NN8 * DIM], F32, tag=tag)
                        # stage[p, ni*768 + c] = src[ni*128 + p, c]  (one DMA)
                        nc.sync.dma_start(
                            stg[:].rearrange("p (b c) -> p b c", c=DIM),
                            src_d.rearrange("(b p) c -> p b c", p=128),
                        )
                        stages.append(stg)
                    for stg, dstT in ((stages[0], enT), (stages[1], decT)):
                        absorb(stg)
                        for ni in range(NN8):
                            x = stg[:, ni * DIM:(ni + 1) * DIM]
                            tp = tpp.tile([128, DIM], F32, tag="tp")
                            for cj in range(NC6):
                                nc.tensor.transpose(
                                    tp[:, cj * 128:(cj + 1) * 128],
                                    x[:, cj * 128:(cj + 1) * 128],
                                    ident[:],
                                )
                            # scatter the 6 transposed blocks to cols cj*1024+ni*128
                            dst = dstT.rearrange("p (c n) -> p c n", n=N)[
                                :, :, ni * 128:(ni + 1) * 128
                            ]
                            nc.vector.tensor_copy(
                                _r(dst), tp.rearrange("p (c n) -> p c n", n=128)
                            )

                # =============================================================
                # Phase B: kT = (en @ Wkv_k + b)^T ;  v = en @ Wkv_v + b
                # =============================================================
                with (
                    tc.tile_pool(name="wkv", bufs=1) as wvp,
                    tc.tile_pool(name="mm", bufs=3, space="PSUM") as mmp,
                ):
                    wk = []
                    wv = []
                    for cj in range(NC6):
                        rawk = io.tile([128, DIM], F32, tag="wraw")
                        nc.sync.dma_start(rawk[:], wkv_d[cj * 128:(cj + 1) * 128, 0:DIM])
                        wkt = wvp.tile([128, DIM], F32, tag=f"wk{cj}")
                        nc.vector.tensor_copy(_r(wkt[:]), rawk[:])
                        wk.append(wkt)
                        rawv = io.tile([128, DIM], F32, tag="wraw")
                        nc.sync.dma_start(rawv[:], wkv_d[cj * 128:(cj + 1) * 128, DIM:2 * DIM])
                        wvt = wvp.tile([128, DIM], F32, tag=f"wv{cj}")
                        nc.vector.tensor_copy(_r(wvt[:]), rawv[:])
                        wv.append(wvt)

                    # kT[d2, n]: lhsT = Wkv_k[c, d2] chunk, rhs = enT[c, n]
                    for dj in range(NC6):
                        for nt in range(NQ2):
                            ps = mmp.tile([128, 512], F32, tag="mm")
                            for cj in range(NC6):
                                nc.tensor.matmul(
                                    ps[:],
                                    _r(wk[cj][:, dj * 128:(dj + 1) * 128]),
                                    _r(enT[:, cj * N + nt * 512: cj * N + nt * 512 + 512]),
                                    start=(cj == 0),
                                    stop=(cj == NC6 - 1),
                                )
                            nc.vector.tensor_scalar_add(
                                _r(kT[:, dj * N + nt * 512: dj * N + nt * 512 + 512]),
                                ps[:],
                                bkvk[:, dj:dj + 1],
                            )

                    # v[n, d2]: lhsT = enT[c, n-chunk], rhs = Wkv_v[c, d2]
                    for ni in range(NN8):
                        ones = vext[ni].rearrange("p (h c) -> p h c", c=HD + 1)[:, :, HD:HD + 1]
                        nc.vector.tensor_copy(
                            _r(ones), onesrc[:].rearrange("p (h c) -> p h c", c=1)
                        )
                        for dt2, base, sz in ((0, 0, 512), (1, 512, 256)):
                            nh = sz // HD
                            ps = mmp.tile([128, 512], F32, tag="mm")
                            for cj in range(NC6):
                                nc.tensor.matmul(
                                    ps[:, 0:sz],
                                    _r(enT[:, cj * N + ni * 128: cj * N + ni * 128 + 128]),
                                    _r(wv[cj][:, base:base + sz]),
                                    start=(cj == 0),
                                    stop=(cj == NC6 - 1),
                                )
                            h0 = base // HD
                            dst = vext[ni].rearrange("p (h c) -> p h c", c=HD + 1)[
                                :, h0:h0 + nh, 0:HD
                            ]
                            nc.vector.tensor_add(
                                _r(dst),
                                ps[:, 0:sz].rearrange("p (h c) -> p h c", c=HD),
                                bkvv[:, base:base + sz].rearrange("p (h c) -> p h c", c=HD),
                            )

            # =================================================================
            # Phase C: attention per head pair
            # =================================================================
            with (
                tc.tile_pool(name="sps", bufs=1, space="PSUM") as spsp,
                tc.tile_pool(name="wps", bufs=1, space="PSUM") as wpsp,
                tc.tile_pool(name="rps", bufs=1, space="PSUM") as rpsp,
                tc.tile_pool(name="epool", bufs=2) as ep,
                tc.tile_pool(name="norm", bufs=2) as npo,
            ):
                for pj in range(NP):
                    hA, hB = 2 * pj, 2 * pj + 1
                    for qt in range(NQ2):
                        q0 = qt * 512
                        wps = {}
                        for hh, tag in ((hA, "wpsA"), (hB, "wpsB")):
                            wps[hh] = wpsp.tile([HD + 1, 512], F32, tag=tag, name=f"{tag}_{pj}_{qt}")
                        for kcp in range(NN8 // 2):
                            sps = {}
                            e = {}
                            for idx, (hh, row0) in enumerate(((hA, 0), (hB, 64))):
                                sps[hh] = spsp.tile(
                                    [128, 1024], F32, tag=("spsA", "spsB")[idx],
                                    name=f"sps{idx}_{pj}_{qt}_{kcp}",
                                )
                                for sub in range(2):
                                    kc = kcp * 2 + sub
                                    # scoresT[k, q] = k_h @ q_h^T ; K = HD = 64
                                    nc.tensor.matmul(
                                        sps[hh][:, sub * 512:sub * 512 + 512],
                                        _r(kT[row0:row0 + 64,
                                              pj * N + kc * 128: pj * N + kc * 128 + 128]),
                                        _r(decT[row0:row0 + 64, pj * N + q0: pj * N + q0 + 512]),
                                        start=True,
                                        stop=True,
                                        tile_position=(row0, 0),
                                    )
                                et = ep.tile([128, 1024], F32, tag=("eA", "eB")[idx])
                                nc.scalar.activation(
                                    _r(et[:]), sps[hh][:],
                                    mybir.ActivationFunctionType.Exp,
                                    scale=SCALE,
                                )
                                e[hh] = et
                            for hh in (hA, hB):
                                for sub in range(2):
                                    kc = kcp * 2 + sub
                                    if kc == 0:
                                        absorb(e[hh])
                                    # waT accum: lhsT = [v_h | 1], rhs = expT
                                    nc.tensor.matmul(
                                        wps[hh][:],
                                        _r(vext[kc][:, hh * (HD + 1): (hh + 1) * (HD + 1)]),
                                        _r(e[hh][:, sub * 512:sub * 512 + 512]),
                                        start=(kc == 0),
                                        stop=(kc == NN8 - 1),
                                    )
                        # normalize: row 64 of wps = softmax denominators for q
                        for hh, row0 in ((hA, 0), (hB, 64)):
                            rt = npo.tile([1, 512], F32, tag="rt", name=f"rt{hh}_{qt}")
                            nc.vector.reciprocal(_r(rt[:]), wps[hh][HD:HD + 1, :])
                            # broadcast r across 64 partitions: ones^T @ r (K=1)
                            rp = rpsp.tile([HD, 512], F32, tag="rps")
                            nc.tensor.matmul(
                                rp[:], _r(ones64[:]), _r(rt[:]), start=True, stop=True
                            )
                            rb = npo.tile([HD, 512], F32, tag="rsbuf")
                            nc.vector.tensor_copy(rb[:], rp[:])
                            nc.vector.tensor_mul(
                                _r(waT[row0:row0 + HD, pj * N + q0: pj * N + q0 + 512]),
                                wps[hh][0:HD, :],
                                rb[:],
                            )

            # =================================================================
            # Phase D: out = waT^T @ Wproj + bproj
            # =================================================================
            with (
                tc.tile_pool(name="dpool", bufs=1) as dp,
                tc.tile_pool(name="pd", bufs=2, space="PSUM") as pdp,
            ):
                wp = []
                for cj in range(NC6):
                    praw = io.tile([128, DIM], F32, tag="praw")
                    nc.sync.dma_start(praw[:], wproj_d[cj * 128:(cj + 1) * 128, :])
                    wpt = dp.tile([128, DIM], F32, tag=f"wp{cj}")
                    nc.vector.tensor_copy(_r(wpt[:]), praw[:])
                    wp.append(wpt)
                for ni in range(NN8):
                    osb = io.tile([128, DIM], F32, tag="osb")
                    for dt2, base, sz in ((0, 0, 512), (1, 512, 256)):
                        ps = pdp.tile([128, 512], F32, tag="pd")
                        for cj in range(NC6):
                            nc.tensor.matmul(
                                ps[:, 0:sz],
                                _r(waT[:, cj * N + ni * 128: cj * N + ni * 128 + 128]),
                                _r(wp[cj][:, base:base + sz]),
                                start=(cj == 0),
                                stop=(cj == NC6 - 1),
                            )
                        nc.vector.tensor_add(
                            osb[:, base:base + sz], ps[:, 0:sz], bprojb[:, base:base + sz]
                        )
                    nc.sync.dma_start(out_d[ni * 128:(ni + 1) * 128, :], osb[:])

    _strip_self_waits(nc)
    _split_excess_waits(nc)
    return nc


# In-order engines don't need to wait on their own completion semaphore
# (program order already serializes same-engine dependencies), but the tile
# framework emits such waits for WAW psum reuse.  Walrus matmul lowering only
# has one sync-wait slot, so drop the redundant self-waits.
_SEM_ENGINE = {"PE": "PE", "DVE": "DVE", "ACT": "Activation", "POOL": "Pool", "SP": "SP"}


def _strip_self_waits(nc):
    for bb in nc.main_func.blocks:
        for inst in bb.instructions:
            si = getattr(inst, "sync_info", None)
            if si is None or not si.on_wait:
                continue
            if inst.opcode == "DMACopy":
                continue
            eng = getattr(inst, "engine", None)
            eng_name = getattr(eng, "name", str(eng))
            kept = []
            for w in si.on_wait:
                sem_eng = _SEM_ENGINE.get(str(w.ant_name).split("_")[0])
                if sem_eng is not None and sem_eng == eng_name:
                    continue
                kept.append(w)
            si.on_wait = kept


# Walrus codegen has limited sync-wait slots per instruction (matmul: 1,
# most others: 2).  Hoist excess waits onto preceding same-engine NOPs —
# engines execute their stream in order, so a wait satisfied by an earlier
# NOP also gates every later instruction on that engine.
def _split_excess_waits(nc):
    import concourse.mybir as mybir
    for bb in nc.main_func.blocks:
        new_insts = []
        for inst in bb.instructions:
            si = getattr(inst, "sync_info", None)
            cap = 1
            if si is not None and len(si.on_wait) > cap:
                excess = si.on_wait[cap:]
                si.on_wait = si.on_wait[:cap]
                for w in excess:
                    nop = mybir.InstNoOp(
                        name=nc.get_next_instruction_name(),
                        engine=inst.engine,
                        bass_nofuse=True,
                        sync_info=mybir.SyncInfo(on_wait=[w], on_update=[]),
                    )
                    nc.register_instruction(nop)
                    new_insts.append(nop)
            new_insts.append(inst)
        bb.instructions[:] = new_insts


def make_const_inputs(bkv, bproj):
    ident = np.eye(128, dtype=np.float32)
    bkv_k = np.ascontiguousarray(
        bkv[:DIM].reshape(NC6, 128).T.astype(np.float32)
    )
    bkv_v = np.tile(bkv[DIM:].astype(np.float32)[None, :], (128, 1))
    bproj_b = np.tile(bproj.astype(np.float32)[None, :], (128, 1))
    return {
        "ident": ident,
        "bkv_k": np.ascontiguousarray(bkv_k),
        "bkv_v": np.ascontiguousarray(bkv_v),
        "bproj_b": np.ascontiguousarray(bproj_b),
    }


def make_in_maps(en, dec, Wkv, bkv, Wproj, bproj):
    consts = make_const_inputs(np.asarray(bkv), np.asarray(bproj))
    in_maps = []
    for i in range(NCORES):
        m = {
            "en": np.ascontiguousarray(np.asarray(en)[i], dtype=np.float32),
            "dec": np.ascontiguousarray(np.asarray(dec)[i], dtype=np.float32),
            "Wkv": np.ascontiguousarray(np.asarray(Wkv), dtype=np.float32),
            "Wproj": np.ascontiguousarray(np.asarray(Wproj), dtype=np.float32),
        }
        m.update(consts)
        in_maps.append(m)
    return in_maps


_NC_CACHE = None


def kernel(en, dec, Wkv, bkv, Wproj, bproj):
    global _NC_CACHE
    if _NC_CACHE is None:
        _NC_CACHE = build_bass()
    in_maps = make_in_maps(en, dec, Wkv, bkv, Wproj, bproj)
    res = run_bass_kernel_spmd(_NC_CACHE, in_maps, list(range(NCORES)))
    out = np.stack([res.results[i]["out"] for i in range(NCORES)], axis=0)
    return out.astype(np.float32)

